# revision 23
# baseline (speedup 1.0000x reference)
"""Bass/Tile TRN2 kernel for nn_Attention_3264175145281.

Computes, for each batch row b:
    energy[s] = encoder_outputs[b, s, :] @ W[0, :512]   (+ const(b), dropped)
    weights   = softmax(energy)
    context   = weights @ encoder_outputs[b]

The reference adds `hidden @ W[0, 512:] + bias` to every energy[s]; that term
is constant along s, and softmax is shift-invariant, so the output does not
depend on it.  We therefore stream encoder_outputs exactly once per core.

Perf structure (v4):
  - Input stream rides the SWDGE (gpsimd) queue and casts f32 -> bf16 inline:
    same HBM read bytes, half the SBUF footprint, 2 MiB per transfer.
  - Output DMAs ride the HWDGE (sync) ring -- a different queue -- so the
    per-row tail never head-of-line-blocks the next row's input stream.
  - The energy pass (x . w per position) is load-balanced across DVE and ACT
    using HW-measured per-op costs:
      DVE waves: fused STT per chunk (mult+accum, 1x, ~766 ns/chunk all-in);
      ACT waves: one DVE tensor_tensor over the 4-chunk wave (bf16 2x mode,
                 ~366 ns/chunk) + four ACT Copy+accum reduces (~1034 ns/chunk
                 on the otherwise idle scalar engine).
    With 3 of 8 waves on ACT both engines stay under the ~82 us DMA stream.

Sharding: batch dim across 8 NeuronCores (4 rows each), W replicated.
"""

import os
import sys

import numpy as np

for _p in ("/opt/trn_rl_repo", os.path.expanduser("~/.axon_site/_ro/trn_rl_repo")):
    if os.path.isdir(_p) and _p not in sys.path:
        sys.path.insert(0, _p)

from contextlib import ExitStack

import concourse.bacc as bacc
import concourse.bass as bass
import concourse.mybir as mybir
import concourse.tile as tile
from concourse.bass_utils import run_bass_kernel_spmd

B, S, ENC = 32, 4096, 512
NCORES = 8
B_LOC = B // NCORES          # 4 batch rows per core
P = 128                      # SBUF partitions
NCH = S // P                 # 32 chunks of 128 positions
GRP = 8                      # chunks per DMA group (2 MiB f32 read per dma)
NGRP = NCH // GRP            # 4 group DMAs per batch row
EGRP = 4                     # chunks per wave (mult/reduce/exp/matmul unit)
NEG = NCH // EGRP            # 8 waves per batch row
# Waves routed to ACT (DVE does only the fused 4-chunk multiply; ACT does the
# four Copy+accum reduces).  Other waves run fully on the DVE as per-chunk
# STTs.  Wave 7 stays on the DVE to keep the kernel tail short.
ACT_WAVES = frozenset({1, 4, 6})
XG_BUFS = 14                 # x_pool depth (1 MiB per slot)
SPLIT_LAST = True            # split the final group's DMA into two halves
SPLIT_ALL_ROWS = False       # split the final group of EVERY row, not just last
                             # (A/B: 104-112 us vs 120-123 us for unsplit)
F32 = mybir.dt.float32
BF16 = mybir.dt.bfloat16


def build_program(n_b: int = B_LOC) -> bass.Bass:
    nc = bacc.Bacc("TRN2", target_bir_lowering=False, debug=False)

    x = nc.dram_tensor("x", [n_b, S, ENC], F32, kind="ExternalInput").ap()
    wenc = nc.dram_tensor("wenc", [1, ENC], F32, kind="ExternalInput").ap()
    out = nc.dram_tensor("out", [n_b, ENC], F32, kind="ExternalOutput").ap()

    with tile.TileContext(nc) as tc, ExitStack() as ctx:
        const_pool = ctx.enter_context(tc.tile_pool(name="const", bufs=1))
        x_pool = ctx.enter_context(tc.tile_pool(name="xg", bufs=XG_BUFS))
        xh_pool = ctx.enter_context(tc.tile_pool(name="xgh", bufs=4))
        scr_pool = ctx.enter_context(tc.tile_pool(name="scr", bufs=8))
        stat_pool = ctx.enter_context(tc.tile_pool(name="stat", bufs=4))
        rs_pool = ctx.enter_context(tc.tile_pool(name="rs", bufs=4 * NEG))
        out_pool = ctx.enter_context(tc.tile_pool(name="outp", bufs=4))
        psum_pool = ctx.enter_context(tc.tile_pool(name="psum", bufs=4, space="PSUM"))

        # w_enc broadcast to all partitions (HWDGE, f32), then replicated
        # x4 along the free dim in bf16 to pair with 4-chunk mult tiles.
        wb32 = const_pool.tile([P, ENC], F32, tag="wb32")
        nc.sync.dma_start(wb32[:], wenc[:, :].broadcast_to([P, ENC]))

        wb4 = const_pool.tile([P, EGRP, ENC], BF16, tag="wb4")
        for k in range(EGRP):
            nc.vector.tensor_copy(wb4[:, k, :], wb32[:])

        ones = const_pool.tile([P, 1], F32, tag="ones")
        nc.gpsimd.memset(ones[:], 1.0)
        dummy = const_pool.tile([P, 1], BF16, tag="dummy")

        def make_tail(b, ctx_psum, z_psum):
            def tail():
                rz = stat_pool.tile([1, 1], F32, tag="rz")
                nc.vector.reciprocal(rz[:], z_psum[:])
                ot = out_pool.tile([1, ENC], F32, tag="ot")
                nc.scalar.activation(
                    ot[:], ctx_psum[:], mybir.ActivationFunctionType.Copy,
                    scale=rz[:],
                )
                nc.sync.dma_start(out[b:b + 1, :], ot[:])
            return tail

        for b in range(n_b):
            # chunk_map[j] = (tile, k) so that tile[:, k, :] is chunk j.
            chunk_map = []
            # Separate energy tiles per producing engine so Tile's dependency
            # tracking never serializes DVE-reduce against ACT-reduce waves.
            energy_d = stat_pool.tile([P, NCH], F32, tag="energy_d")
            energy_a = stat_pool.tile([P, NCH], F32, tag="energy_a")
            p_t = stat_pool.tile([P, NCH], BF16, tag="p")
            ctx_psum = psum_pool.tile([1, ENC], F32, tag="ctx")
            z_psum = psum_pool.tile([1, 1], F32, tag="z")

            for g in range(NGRP):
                # s = g*P*GRP + p*GRP + k: each partition reads one contiguous
                # 16 KiB f32 run from DRAM (2 MiB per dma_start); SWDGE casts
                # to bf16 inline (1 MiB landed in SBUF).  The very last group
                # can be fetched as two half-tiles so the second-to-last
                # wave's data (and compute) starts half a transfer earlier.
                c0 = g * GRP
                split_here = (g == NGRP - 1 if SPLIT_ALL_ROWS
                              else (b == n_b - 1 and g == NGRP - 1))
                halves = 2 if (SPLIT_LAST and split_here) else 1
                sub = GRP // halves
                for h in range(halves):
                    if halves == 1:
                        gx = x_pool.tile([P, sub, ENC], BF16, tag="gx")
                    else:
                        gx = xh_pool.tile([P, sub, ENC], BF16, tag="gxh")
                    s0 = (c0 + h * sub) * P
                    src = x[b, s0:s0 + sub * P, :]
                    nc.gpsimd.dma_start(
                        gx[:], src.rearrange("(p k) e -> p k e", p=P))
                    for k in range(sub):
                        chunk_map.append((gx, k))

                for wv in range(GRP // EGRP):          # 2 waves per group
                    e = g * (GRP // EGRP) + wv         # wave index 0..NEG-1
                    j0 = e * EGRP
                    k0 = wv * EGRP
                    on_act = e in ACT_WAVES
                    energy = energy_a if on_act else energy_d

                    last_wave = (b == n_b - 1 and e == NEG - 1)

                    if on_act:
                        # multiply on DVE (one bf16 TT over the wave, 2x mode)
                        # (GPSIMD tensor_mul measured ~7x slower and it
                        # head-of-line-blocks the SWDGE DMA dispatches.)
                        mt, mk = chunk_map[j0]
                        scr4 = scr_pool.tile([P, EGRP, ENC], BF16, tag="scr4")
                        nc.vector.tensor_mul(
                            scr4[:], mt[:, mk:mk + EGRP, :], wb4[:])
                        # reduce on ACT: energy[:, j0+i] = sum_e scr4[:, i, e]
                        for i in range(EGRP):
                            nc.scalar.activation(
                                dummy[:].broadcast_to([P, ENC]), scr4[:, i, :],
                                mybir.ActivationFunctionType.Copy,
                                accum_out=energy[:, j0 + i:j0 + i + 1],
                            )
                    else:
                        # fused mult+accum per chunk on DVE
                        for i in range(EGRP):
                            scr = scr_pool.tile([P, ENC], BF16, tag="scr1")
                            st, sk = chunk_map[j0 + i]
                            nc.vector.scalar_tensor_tensor(
                                out=scr[:],
                                in0=st[:, sk, :],
                                scalar=1.0,
                                in1=wb4[:, 0, :],
                                op0=mybir.AluOpType.mult,
                                op1=mybir.AluOpType.mult,
                                accum_out=energy[:, j0 + i:j0 + i + 1],
                            )
                            if last_wave:
                                # kernel tail: per-chunk exp + matmul so the
                                # final dependency chain is as short as
                                # possible after the last DMA byte lands.
                                jj = j0 + i
                                rowsum = rs_pool.tile([P, 1], F32, tag="rowsum")
                                nc.scalar.activation(
                                    p_t[:, jj:jj + 1], energy[:, jj:jj + 1],
                                    mybir.ActivationFunctionType.Exp,
                                    accum_out=rowsum[:],
                                )
                                nc.tensor.matmul(
                                    z_psum[:], rowsum[:], ones[:],
                                    start=False, stop=(jj == NCH - 1),
                                )
                                ct, ck = chunk_map[jj]
                                nc.tensor.matmul(
                                    ctx_psum[:],
                                    p_t[:, jj:jj + 1],
                                    ct[:, ck, :],
                                    start=False,
                                    stop=(jj == NCH - 1),
                                )

                    if last_wave:
                        continue

                    # softmax numerator + denominator + context accumulation
                    rowsum = rs_pool.tile([P, 1], F32, tag="rowsum")
                    nc.scalar.activation(
                        p_t[:, j0:j0 + EGRP], energy[:, j0:j0 + EGRP],
                        mybir.ActivationFunctionType.Exp,
                        accum_out=rowsum[:],
                    )
                    nc.tensor.matmul(
                        z_psum[:], rowsum[:], ones[:],
                        start=(e == 0), stop=(e == NEG - 1),
                    )
                    for jj in range(j0, j0 + EGRP):
                        ct, ck = chunk_map[jj]
                        nc.tensor.matmul(
                            ctx_psum[:],
                            p_t[:, jj:jj + 1],
                            ct[:, ck, :],
                            start=(jj == 0),
                            stop=(jj == NCH - 1),
                        )

            make_tail(b, ctx_psum, z_psum)()

    nc.compile()
    return nc


_CACHED_NC = None


def _get_nc() -> bass.Bass:
    global _CACHED_NC
    if _CACHED_NC is None:
        _CACHED_NC = build_program()
    return _CACHED_NC


def run(inputs: dict, trace: bool = False, **kw):
    """Shard inputs, run on 8 cores, return (full_output, BassKernelResults)."""
    x_full = np.ascontiguousarray(np.asarray(inputs["encoder_outputs"], dtype=np.float32))
    w_full = np.ascontiguousarray(np.asarray(inputs["W"], dtype=np.float32))
    wenc = np.ascontiguousarray(w_full[:, :ENC])

    nc = _get_nc()
    in_maps = [
        {"x": np.ascontiguousarray(x_full[c * B_LOC:(c + 1) * B_LOC]), "wenc": wenc}
        for c in range(NCORES)
    ]
    res = run_bass_kernel_spmd(nc, in_maps, list(range(NCORES)), trace=trace, **kw)
    out = np.concatenate([res.results[c]["out"] for c in range(NCORES)], axis=0)
    return out.astype(np.float32), res


def kernel(encoder_outputs, hidden, W, b):
    out, _ = run({"encoder_outputs": encoder_outputs, "W": W})
    return out
